# revision 7
# baseline (speedup 1.0000x reference)
"""GAT (3-layer, 2-branch) Bass/Trainium2 kernel for nn_GAT_6854767804552. v2.

Design: per-core dst ownership by graph ranges; per-layer allgathered bf16
node tables (h' = h@Q with al_s folded into last col); edge aggregation via
dma_gather (two overlapping 32768-row windows to fit int16 indices) +
DVE broadcast-scale + PE identity-matmul PSUM accumulation.
"""
import math
import numpy as np
import ml_dtypes

import concourse.bass as bass
import concourse.mybir as mybir
import concourse.tile as tile
from concourse import bacc
from contextlib import ExitStack
from concourse.bass_utils import run_bass_kernel_spmd
from concourse.masks import make_identity

F32 = mybir.dt.float32
BF16 = mybir.dt.bfloat16
I16 = mybir.dt.int16
AF = mybir.ActivationFunctionType
OP = mybir.AluOpType

P = 128
R = 8
N_NODES = 50000
N_GRAPHS = 2048
GPC = N_GRAPHS // R  # 256
NEG = 0.2
DIMS = [(7, 128), (128, 128), (128, 64)]
W1CAP = 32768  # int16 index window size
GCAP = 8       # max slots per gather call (1024 idxs)


# ----------------------------------------------------------------- host planning

def _wrap16(flat):
    flat = np.asarray(flat, dtype=np.int64)
    n = len(flat)
    assert n % 16 == 0
    assert flat.max() <= 32767 and flat.min() >= 0, (flat.min(), flat.max())
    blk = flat.reshape(-1, 16).T.astype(np.int16)
    return np.tile(blk, (8, 1))


def _householder_q(a):
    D = len(a)
    na = np.linalg.norm(a)
    u0 = a / na
    e = np.zeros(D); e[-1] = 1.0
    v = e - u0
    nv = np.linalg.norm(v)
    if nv < 1e-7:
        H = np.eye(D)
    else:
        v = v / nv
        H = np.eye(D) - 2.0 * np.outer(v, v)
    Q = H.copy()
    Q[:, -1] = a
    S = np.ones(D); S[-1] = 1.0 / na
    Qinv = (S[:, None] * H.T)
    return Q.astype(np.float64), Qinv.astype(np.float64)


def _plan_branch(edge_index, bounds, own, NPAD):
    """Two-window slot packing. Returns pos_of, node_at, C1, C2, idx streams."""
    NB = NPAD // P
    NROWS = R * NPAD
    B2 = NROWS - W1CAP  # window2 base
    B3 = (NROWS - W1CAP) // 2  # window3 base: covers [B3, B3+W1CAP)
    src = np.concatenate([edge_index[0], np.arange(N_NODES, dtype=np.int64)])
    dst = np.concatenate([edge_index[1], np.arange(N_NODES, dtype=np.int64)])

    deg = np.bincount(dst, minlength=N_NODES)

    pos_of = np.zeros(N_NODES, dtype=np.int64)
    node_at = np.full((R, NPAD), -1, dtype=np.int64)
    for r in range(R):
        ids = np.arange(bounds[r], bounds[r + 1])
        order = ids[np.argsort(-deg[ids], kind="stable")]
        pos_of[order] = np.arange(len(order))
        node_at[r, :len(order)] = order

    row = own * NPAD + pos_of          # global table row per node
    srow = row[src]                    # per-edge src row
    # categories by allowed windows: 0={w1} 1={w1,w3} 2={w1,w2,w3} 3={w2,w3} 4={w2}
    cat = np.select(
        [srow < B3, srow < B2, srow < W1CAP, srow < B3 + W1CAP],
        [0, 1, 2, 3], default=4).astype(np.int64)

    prng = np.random.default_rng(12345)

    e_own = own[dst]
    j_all = pos_of[dst]

    # per-node counts per category
    n_cnt = np.zeros((5, N_NODES), dtype=np.int64)
    for cval in range(5):
        n_cnt[cval] = np.bincount(dst[cat == cval], minlength=N_NODES)

    C1 = np.zeros(NB, dtype=np.int64)
    C2 = np.zeros(NB, dtype=np.int64)
    C3 = np.zeros(NB, dtype=np.int64)
    for b in range(NB):
        best = None
        # nodes in this block across all cores
        ids = node_at[:, b * P:(b + 1) * P].ravel()
        ids = ids[ids >= 0]
        n0 = n_cnt[0][ids]; n1 = n_cnt[1][ids]; n2 = n_cnt[2][ids]
        n3 = n_cnt[3][ids]; n4 = n_cnt[4][ids]
        deg = n0 + n1 + n2 + n3 + n4
        lo1, lo2 = int(n0.max()), int(n4.max())
        lo12 = int((n0 + n4).max())
        for t1 in range(lo1, lo1 + 10):
            for t2 in range(lo2, lo2 + 10):
                if t1 + t2 < lo12:
                    continue
                t3 = int(np.maximum.reduce([
                    n0 + n1 - t1, n4 + n3 - t2, deg - t1 - t2,
                    np.zeros_like(deg)]).max())
                cost = (math.ceil(t1 / GCAP) + math.ceil(t2 / GCAP)
                        + math.ceil(t3 / GCAP), t1 + t2 + t3)
                if best is None or cost < best[0]:
                    best = (cost, t1, t2, t3)
        C1[b], C2[b], C3[b] = best[1], best[2], best[3]

    # per-core slot streams: per block [C1 w1][C2 w2][C3 w3] slots
    ia_list = []
    im_list = []
    for r in range(R):
        m = e_own == r
        j_r, cat_r, srow_r = j_all[m], cat[m], srow[m]
        parts = []
        mparts = []
        for b in range(NB):
            t1, t2, t3 = int(C1[b]), int(C2[b]), int(C3[b])
            mb = (j_r // P) == b
            j_b = j_r[mb]; cat_b = cat_r[mb]; srow_b = srow_r[mb]
            # sort by (dst pos, category): cat0 first ... cat4 last per dst
            o = np.lexsort((cat_b, j_b))
            j_b, cat_b, srow_b = j_b[o], cat_b[o], srow_b[o]
            starts = np.searchsorted(j_b, np.arange(b * P, (b + 1) * P))
            rank = np.arange(len(j_b)) - starts[j_b - b * P]
            part = j_b % P
            # per-dst category counts and window quotas
            ids_p = node_at[r, b * P:(b + 1) * P]
            okp = ids_p >= 0
            mm = [np.where(okp, n_cnt[cv][np.clip(ids_p, 0, None)], 0)
                  for cv in range(5)]
            a1 = np.minimum(mm[1], t1 - mm[0])
            a21 = np.minimum(mm[2], t1 - mm[0] - a1)
            b3 = np.minimum(mm[3], t2 - mm[4])
            b2 = np.minimum(mm[2] - a21, t2 - mm[4] - b3)
            # per-dst rank boundaries (cat order: 0,1,2,3,4)
            bnd1 = mm[0] + a1
            bnd2 = mm[0] + mm[1]
            bnd3 = bnd2 + a21
            bnd4 = bnd3 + b2
            bnd5 = bnd2 + mm[2]
            bnd6 = bnd5 + b3
            bnd7 = bnd5 + mm[3]
            B = lambda a: a[part]
            in1 = (rank < B(bnd1)) | ((rank >= B(bnd2)) & (rank < B(bnd3)))
            in2 = (((rank >= B(bnd3)) & (rank < B(bnd4)))
                   | ((rank >= B(bnd5)) & (rank < B(bnd6)))
                   | (rank >= B(bnd7)))
            in3 = ~(in1 | in2)
            r1 = np.where(rank < B(bnd1), rank, B(bnd1) + rank - B(bnd2))
            r2_ = np.where(rank < B(bnd4), rank - B(bnd3),
                           np.where(rank < B(bnd6),
                                    B(bnd4) - B(bnd3) + rank - B(bnd5),
                                    B(bnd4) - B(bnd3) + B(bnd6) - B(bnd5)
                                    + rank - B(bnd7)))
            r3 = np.where(rank < B(bnd2), rank - B(bnd1),
                          np.where(rank < B(bnd5),
                                   B(bnd2) - B(bnd1) + rank - B(bnd4),
                                   B(bnd2) - B(bnd1) + B(bnd5) - B(bnd4)
                                   + rank - B(bnd6)))
            for tw, inw, rw, base in ((t1, in1, r1, 0), (t2, in2, r2_, B2),
                                      (t3, in3, r3, B3)):
                if tw == 0:
                    assert not inw.any()
                    continue
                arr = prng.integers(0, W1CAP, (tw, P)).astype(np.int64)
                v = np.zeros((tw, P), dtype=bool)
                assert (rw[inw] >= 0).all() and (rw[inw] < tw).all()
                rel = srow_b[inw] - base
                assert (rel >= 0).all() and (rel < W1CAP).all()
                arr[rw[inw], part[inw]] = rel
                v[rw[inw], part[inw]] = True
                parts.append(arr.ravel())
                mparts.append(v.T)
        ia_list.append(np.concatenate(parts))
        im_list.append(np.concatenate(mparts, axis=1).astype(np.float32)
                       .astype(ml_dtypes.bfloat16))

    return dict(pos_of=pos_of, node_at=node_at, C1=C1, C2=C2, C3=C3,
                ia=ia_list, im=im_list)


def _plan(inputs):
    batch = np.asarray(inputs["batch"], dtype=np.int64)
    bounds = np.searchsorted(batch, np.arange(R + 1) * GPC)
    L = np.diff(bounds)
    own = np.repeat(np.arange(R), L)
    NB = math.ceil((L.max() + 1) / P)
    NPAD = NB * P
    assert R * NPAD > W1CAP and R * NPAD - W1CAP <= W1CAP

    b1 = _plan_branch(np.asarray(inputs["edge_index1"], np.int64), bounds, own, NPAD)
    b2 = _plan_branch(np.asarray(inputs["edge_index2"], np.int64), bounds, own, NPAD)

    sizes = np.bincount(batch, minlength=N_GRAPHS)
    gb_bounds = np.concatenate([[0], np.cumsum(sizes)])
    NGB = GPC // P  # 2
    gorder = np.zeros((R, GPC), dtype=np.int64)
    PC = np.zeros(NGB, dtype=np.int64)
    for r in range(R):
        gl = np.arange(r * GPC, (r + 1) * GPC)
        go = gl[np.argsort(-sizes[gl], kind="stable")]
        gorder[r] = go
        PC = np.maximum(PC, sizes[go].reshape(NGB, P).max(axis=1))

    def pool_stream(plan):
        out = []
        for r in range(R):
            parts = []
            for gb in range(NGB):
                nb = int(PC[gb])
                arr = np.full((nb, P), NPAD, dtype=np.int64)  # pad -> zero row
                for p in range(P):
                    g = gorder[r, gb * P + p]
                    mem = np.arange(gb_bounds[g], gb_bounds[g + 1])
                    arr[:len(mem), p] = plan["pos_of"][mem]
                parts.append(arr.ravel())
            out.append(np.concatenate(parts))
        return out

    return dict(bounds=bounds, L=L, own=own, NB=NB, NPAD=NPAD,
                b1=b1, b2=b2, sizes=sizes, gorder=gorder, PC=PC,
                ip1=pool_stream(b1), ip2=pool_stream(b2))


def _weights_fold(inputs):
    out = []
    for l in range(1, 4):
        W = np.asarray(inputs[f"W{l}"], np.float64)
        a_s = np.asarray(inputs[f"as{l}"], np.float64)
        a_d = np.asarray(inputs[f"ad{l}"], np.float64)
        b = np.asarray(inputs[f"b{l}"], np.float64)
        Q, Qinv = _householder_q(a_s)
        Wr = W @ Q
        Waug = np.concatenate([Wr, (W @ a_d)[:, None]], axis=1)
        out.append(dict(Waug=Waug.astype(np.float32),
                        Qinv=Qinv.astype(np.float32),
                        bcol=b.astype(np.float32)[:, None]))
    return out


# ----------------------------------------------------------------- device build

def _build(meta):
    import os
    MAXL = int(os.environ.get("GAT_MAXL", "3"))
    NBR = int(os.environ.get("GAT_BR", "2"))

    NB, NPAD = meta["NB"], meta["NPAD"]
    NROWS = R * NPAD
    B2 = NROWS - W1CAP
    B3 = (NROWS - W1CAP) // 2
    C1 = {1: meta["C11"], 2: meta["C12"]}
    C2 = {1: meta["C21"], 2: meta["C22"]}
    C3 = {1: meta["C31"], 2: meta["C32"]}
    PC = meta["PC"]
    NGB = len(PC)
    KTOT = {br: int(np.sum(C1[br]) + np.sum(C2[br]) + np.sum(C3[br]))
            for br in (1, 2)}
    PK = int(sum(PC))
    CMAXB = {br: int(np.max(C1[br] + C2[br] + C3[br])) for br in (1, 2)}
    CMAX = max(CMAXB[1], CMAXB[2], int(np.max(PC)))

    nc = bacc.Bacc("TRN2", target_bir_lowering=False, num_swdge_queues=4)
    qc = [0]

    def gq():
        qc[0] += 1
        return qc[0] % 4

    # ---------------- inputs
    def din(name, shape, dt=F32):
        return nc.dram_tensor(name, list(shape), dt, kind="ExternalInput")

    xT_in = {1: din("x1T", (7, NPAD)), 2: din("x2T", (7, NPAD))}
    ia_in = {1: din("ia1", (P, KTOT[1] * 8), I16), 2: din("ia2", (P, KTOT[2] * 8), I16)}
    im_in = {1: din("im1", (P, KTOT[1]), BF16), 2: din("im2", (P, KTOT[2]), BF16)}
    ip_in = {1: din("ip1", (P, PK * 8), I16), 2: din("ip2", (P, PK * 8), I16)}
    xn_in = {1: din("xn1T", (16, GPC)), 2: din("xn2T", (16, GPC))}
    invc_in = din("invc", (P, NGB))
    Wa_in = [din(f"Wa{l}", (DIMS[l - 1][0], DIMS[l - 1][1] + 1)) for l in (1, 2, 3)]
    Qi_in = [din(f"Qi{l}", (DIMS[l - 1][1], DIMS[l - 1][1])) for l in (1, 2, 3)]
    bc_in = [din(f"bc{l}", (DIMS[l - 1][1], 1)) for l in (1, 2, 3)]
    linW_in = din("linW", (80, 64))
    linb_in = din("linb", (P, 64))
    padrow_in = {l: din(f"padrow{l}", (1, 128), BF16) for l in (1, 2, 3)}
    o_out = {1: nc.dram_tensor("o1", [GPC, 64], F32, kind="ExternalOutput"),
             2: nc.dram_tensor("o2", [GPC, 64], F32, kind="ExternalOutput")}

    with tile.TileContext(nc) as tc, ExitStack() as ctx:
        cst = ctx.enter_context(tc.tile_pool(name="cst", bufs=1))
        sb = ctx.enter_context(tc.tile_pool(name="sb", bufs=2))
        gpool = ctx.enter_context(tc.tile_pool(name="gp", bufs=6))
        hwp = ctx.enter_context(tc.tile_pool(name="hw", bufs=2))
        ipool = ctx.enter_context(tc.tile_pool(name="ip", bufs=6))
        ps = ctx.enter_context(tc.tile_pool(name="ps", bufs=2, space="PSUM"))
        dr = ctx.enter_context(tc.tile_pool(name="dr", bufs=1, space="DRAM"))

        ident = cst.tile([P, P], F32)
        make_identity(nc, ident[:])
        identb = cst.tile([P, P], BF16)
        make_identity(nc, identb[:])
        Wa_sb, Qi_sb, bc_sb = [], [], []
        for l in range(3):
            w = cst.tile([DIMS[l][0], DIMS[l][1] + 1], F32, name=f"wa{l}")
            nc.sync.dma_start(out=w[:], in_=Wa_in[l][:])
            Wa_sb.append(w)
            q = cst.tile([DIMS[l][1], DIMS[l][1]], F32, name=f"qi{l}")
            nc.sync.dma_start(out=q[:], in_=Qi_in[l][:])
            Qi_sb.append(q)
            b = cst.tile([DIMS[l][1], 1], F32, name=f"bcl{l}")
            nc.sync.dma_start(out=b[:], in_=bc_in[l][:])
            bc_sb.append(b)
        linW_sb = cst.tile([80, 64], F32)
        nc.sync.dma_start(out=linW_sb[:], in_=linW_in[:])
        linb_sb = cst.tile([P, 64], F32)
        nc.sync.dma_start(out=linb_sb[:], in_=linb_in[:])
        invc_sb = cst.tile([P, NGB], F32)
        nc.sync.dma_start(out=invc_sb[:], in_=invc_in[:])
        padr_sb = {}
        for l in (1, 2, 3):
            t = cst.tile([1, 128], BF16, name=f"padr{l}")
            nc.sync.dma_start(out=t[:], in_=padrow_in[l][:])
            padr_sb[l] = t

        branches = (1, 2)[:NBR]

        ag_in = {}
        ald = {}
        tblf = {}

        def emit_allgather(br):
            t = dr.tile([NROWS, 128], BF16, tag=f"tblf{br}", addr_space="Shared")
            nc.gpsimd.collective_compute(
                "AllGather", OP.bypass, replica_groups=[list(range(R))],
                ins=[ag_in[br][:]], outs=[t[:]])
            tblf[br] = t

        # --- L1 table build; AG each branch as soon as its table is ready
        for br in branches:
            x1T = sb.tile([7, NPAD], F32, tag=f"xT{br}", bufs=1)
            nc.sync.dma_start(out=x1T[:], in_=xT_in[br][:])
            ag = dr.tile([NPAD, 128], BF16, tag=f"agin{br}")
            al = sb.tile([P, NB], F32, tag=f"ald{br}", bufs=2)
            for b in range(NB):
                ps1 = ps.tile([P, 136], F32, tag="psA")
                nc.tensor.matmul(ps1[:, :129], x1T[:, b * P:(b + 1) * P],
                                 Wa_sb[0][:], start=True, stop=True)
                sb1 = sb.tile([P, 128], BF16, tag="sb1")
                nc.scalar.copy(out=sb1[:], in_=ps1[:, :128])
                nc.vector.tensor_copy(out=al[:, b:b + 1], in_=ps1[:, 128:129])
                nc.sync.dma_start(out=ag[b * P:(b + 1) * P, :], in_=sb1[:])
            nc.sync.dma_start(out=ag[NPAD - 1:NPAD, :], in_=padr_sb[1][:])
            ag_in[br] = ag
            ald[br] = al
            emit_allgather(br)

        tbl3p = {}

        def emit_blocks(br, l):
            D = DIMS[l - 1][1]
            CA1, CA2, CA3 = C1[br], C2[br], C3[br]
            tw1 = tblf[br][0:W1CAP, :]
            tw2 = tblf[br][B2:NROWS, :]
            tw3 = tblf[br][B3:B3 + W1CAP, :]
            if l < 3:
                Dn = DIMS[l][1]
                ag = dr.tile([NPAD, 128], BF16, tag=f"agin{br}")
                ald_next = sb.tile([P, NB], F32, tag=f"ald{br}", bufs=2)
            else:
                t3 = dr.tile([NPAD + 1, 64], F32, tag=f"tbl3p{br}")
                z64 = sb.tile([1, 64], F32, tag="z64")
                nc.vector.memset(z64[:], 0.0)
                nc.sync.dma_start(out=t3[NPAD:NPAD + 1, :], in_=z64[:])
                tbl3p[br] = t3

            def fetch(b, off):
                c1, c2, c3 = int(CA1[b]), int(CA2[b]), int(CA3[b])
                C = c1 + c2 + c3
                G = gpool.tile([P, CMAX, 128], BF16, tag="G")
                iat = ipool.tile([P, CMAX * 8], I16, tag="iat")
                nc.sync.dma_start(out=iat[:, :C * 8],
                                  in_=ia_in[br][:, off * 8:(off + C) * 8])
                mk = ipool.tile([P, CMAX], BF16, tag="mk")
                nc.sync.dma_start(out=mk[:, :C], in_=im_in[br][:, off:off + C])
                for cbase, cn_w, tw in ((0, c1, tw1), (c1, c2, tw2),
                                        (c1 + c2, c3, tw3)):
                    for c0 in range(0, cn_w, GCAP):
                        cn = min(GCAP, cn_w - c0)
                        nc.gpsimd.dma_gather(
                            out_ap=G[:, cbase + c0:cbase + c0 + cn, :],
                            in_ap=tw,
                            idxs_ap=iat[:, (cbase + c0) * 8:(cbase + c0 + cn) * 8],
                            num_idxs=cn * P, num_idxs_reg=cn * P,
                            elem_size=128, queue_num=gq())

                # e = lrelu(al_s + al_d); strided extract on Scalar engine,
                # lrelu on DVE (contiguous, cheap), exp on Scalar
                e0 = sb.tile([P, CMAX], F32, tag="e0")
                nc.scalar.activation(e0[:, :C], G[:, :C, D - 1], AF.Identity,
                                     bias=ald[br][:, b:b + 1])
                ex = sb.tile([P, CMAX], F32, tag="ex")
                nc.vector.tensor_scalar_max(ex[:, :C], e0[:, :C], 0.0)
                e2 = sb.tile([P, CMAX], F32, tag="e2")
                nc.vector.tensor_scalar(e2[:, :C], e0[:, :C], 0.0, NEG,
                                        op0=OP.min, op1=OP.mult)
                nc.vector.tensor_tensor(out=e0[:, :C], in0=ex[:, :C],
                                        in1=e2[:, :C], op=OP.add)
                w_t = sb.tile([P, CMAX], F32, tag="w_t")
                nc.scalar.activation(w_t[:, :C], e0[:, :C], AF.Exp)
                wb = sb.tile([P, CMAX], BF16, tag="wb")
                nc.vector.tensor_tensor(out=wb[:, :C], in0=w_t[:, :C],
                                        in1=mk[:, :C], op=OP.mult)
                den = sb.tile([P, 1], F32, tag="den")
                nc.vector.tensor_reduce(out=den[:, :1], in_=wb[:, :C],
                                        axis=mybir.AxisListType.X, op=OP.add)
                return (b, C, G, wb, den)

            def consume(st):
                b, C, G, wb, den = st
                # scale + accumulate; split scaling DVE (first k) / Scalar (rest)
                Hw = hwp.tile([P, CMAX, 128], BF16, tag="Hw")
                k = C if C < 9 else (2 * C) // 3
                nc.vector.tensor_tensor(
                    out=Hw[:, :k, :D], in0=G[:, :k, :D],
                    in1=wb[:, :k].unsqueeze(2).to_broadcast((P, k, D)),
                    op=OP.mult)
                if k < C:
                    wm = sb.tile([P, CMAX], F32, tag="wm")
                    nc.vector.tensor_copy(out=wm[:, k:C], in_=wb[:, k:C])
                    for c in range(k, C):
                        nc.scalar.activation(Hw[:, c, :D], G[:, c, :D],
                                             AF.Copy, scale=wm[:, c:c + 1])
                acc = ps.tile([P, 136], F32, tag="psA")
                for c in range(C):
                    nc.tensor.matmul(acc[:, :D], identb[:], Hw[:, c, :D],
                                     start=(c == 0), stop=(c == C - 1))

                rcp = sb.tile([P, 1], F32, tag="rcp")
                nc.vector.tensor_scalar_add(rcp[:], den[:], 1e-30)
                nc.vector.reciprocal(rcp[:], rcp[:])
                z = sb.tile([P, D], F32, tag="zt")
                nc.vector.tensor_scalar_mul(z[:], acc[:, :D], rcp[:, 0:1])

                psT = ps.tile([P, 136], F32, tag="psB")
                nc.tensor.transpose(psT[:D, :P], z[:], ident[:])
                zT = sb.tile([D, P], F32, tag="zT")
                nc.scalar.copy(out=zT[:], in_=psT[:D, :P])
                psU = ps.tile([P, 136], F32, tag="psC")
                nc.tensor.matmul(psU[:D, :P], Qi_sb[l - 1][:], zT[:],
                                 start=True, stop=True)
                m_t = sb.tile([D, P], F32, tag="m_t")
                nc.vector.tensor_scalar(m_t[:], psU[:D, :P], bc_sb[l - 1][:, 0:1],
                                        0.0, op0=OP.add, op1=OP.min)
                r_t = sb.tile([D, P], F32, tag="r_t")
                nc.vector.tensor_scalar(r_t[:], psU[:D, :P], bc_sb[l - 1][:, 0:1],
                                        0.0, op0=OP.add, op1=OP.max)
                u_t = sb.tile([D, P], F32, tag="u_t")
                nc.scalar.activation(u_t[:], m_t[:], AF.Exp)
                xT_new = sb.tile([D, P], F32, tag="xTn")
                nc.vector.scalar_tensor_tensor(
                    out=xT_new[:], in0=u_t[:], scalar=-1.0, in1=r_t[:],
                    op0=OP.add, op1=OP.add)

                if l < 3:
                    ps2 = ps.tile([P, 136], F32, tag="psD")
                    nc.tensor.matmul(ps2[:, :Dn + 1], xT_new[:], Wa_sb[l][:],
                                     start=True, stop=True)
                    sb2 = sb.tile([P, 128], BF16, tag="sb2")
                    nc.scalar.copy(out=sb2[:, :Dn], in_=ps2[:, :Dn])
                    nc.vector.tensor_copy(out=ald_next[:, b:b + 1],
                                          in_=ps2[:, Dn:Dn + 1])
                    nc.sync.dma_start(out=ag[b * P:(b + 1) * P, :Dn],
                                      in_=sb2[:, :Dn])
                else:
                    psV = ps.tile([P, 136], F32, tag="psB")
                    nc.tensor.transpose(psV[:P, :64], xT_new[:], ident[:64, :64])
                    sb4 = sb.tile([P, 64], F32, tag="sb4")
                    nc.scalar.copy(out=sb4[:], in_=psV[:P, :64])
                    nc.sync.dma_start(out=t3[b * P:(b + 1) * P, :], in_=sb4[:])

            # software pipeline: block b's gathers+weights issue before
            # block b-1's scale/accumulate so e0/exp stay ahead of the
            # deferred scalar scales in the Act queue
            off = 0
            prev = None
            for b in range(NB):
                st = fetch(b, off)
                off += st[1]
                if prev is not None:
                    consume(prev)
                prev = st
            consume(prev)

            if l < 3:
                nc.sync.dma_start(out=ag[NPAD - 1:NPAD, :], in_=padr_sb[l + 1][:])
                ag_in[br] = ag
                ald[br] = ald_next

        def emit_pool(br):
            xnT = sb.tile([16, GPC], F32, tag="xnT")
            nc.sync.dma_start(out=xnT[:], in_=xn_in[br][:])
            t3 = tbl3p[br]
            offP = 0
            for gb in range(NGB):
                pc = int(PC[gb])
                Gp = gpool.tile([P, CMAX, 128], BF16, tag="G")
                Gpf = Gp[:].bitcast(F32)
                ipt = ipool.tile([P, CMAX * 8], I16, tag="iat")
                nc.sync.dma_start(out=ipt[:, :pc * 8],
                                  in_=ip_in[br][:, offP * 8:(offP + pc) * 8])
                for c0 in range(0, pc, GCAP):
                    cn = min(GCAP, pc - c0)
                    nc.gpsimd.dma_gather(
                        out_ap=Gpf[:, c0:c0 + cn, :64], in_ap=t3[:],
                        idxs_ap=ipt[:, c0 * 8:(c0 + cn) * 8],
                        num_idxs=cn * P, num_idxs_reg=cn * P,
                        elem_size=64, queue_num=gq())
                offP += pc

                accs = sb.tile([P, 64], F32, tag="accs")
                nc.vector.tensor_copy(out=accs[:], in_=Gpf[:, 0, :64])
                for c in range(1, pc):
                    nc.vector.tensor_tensor(out=accs[:], in0=accs[:],
                                            in1=Gpf[:, c, :64], op=OP.add)
                nc.vector.tensor_scalar_mul(accs[:], accs[:],
                                            invc_sb[:, gb:gb + 1])

                psP = ps.tile([P, 136], F32, tag="psB")
                nc.tensor.transpose(psP[:64, :P], accs[:], ident[:])
                lhsT = sb.tile([80, P], F32, tag="lhsT")
                nc.scalar.copy(out=lhsT[:64, :], in_=psP[:64, :P])
                nc.sync.dma_start(out=lhsT[64:80, :],
                                  in_=xnT[:, gb * P:(gb + 1) * P])
                psO = ps.tile([P, 136], F32, tag="psC")
                nc.tensor.matmul(psO[:, :64], lhsT[:], linW_sb[:],
                                 start=True, stop=True)
                o_sb = sb.tile([P, 64], F32, tag="o_sb")
                nc.vector.tensor_tensor(out=o_sb[:], in0=psO[:, :64],
                                        in1=linb_sb[:], op=OP.add)
                nc.sync.dma_start(out=o_out[br][gb * P:(gb + 1) * P, :], in_=o_sb[:])

        # schedule: blocks of one branch overlap the other's allgather
        for l in range(1, 1 + MAXL):
            for br in branches:
                emit_blocks(br, l)
                if l < MAXL:
                    emit_allgather(br)
        if MAXL == 3:
            for br in branches:
                emit_pool(br)
        else:
            z0 = sb.tile([P, 64], F32, tag="o_sb")
            nc.vector.memset(z0[:], 0.0)
            for br in branches:
                for gb in range(NGB):
                    nc.sync.dma_start(out=o_out[br][gb * P:(gb + 1) * P, :], in_=z0[:])
        for br in (1, 2)[NBR:]:
            z0 = sb.tile([P, 64], F32, tag="o_sb")
            nc.vector.memset(z0[:], 0.0)
            for gb in range(NGB):
                nc.sync.dma_start(out=o_out[br][gb * P:(gb + 1) * P, :], in_=z0[:])

    nc.compile()
    return nc


# ----------------------------------------------------------------- entry point

_CACHE = {}
LAST_RES = None
LAST_RUN_S = None


def kernel(**inputs):
    plan = _plan(inputs)
    NB, NPAD = plan["NB"], plan["NPAD"]
    wf = _weights_fold(inputs)

    meta = dict(NB=NB, NPAD=NPAD,
                C11=plan["b1"]["C1"], C21=plan["b1"]["C2"], C31=plan["b1"]["C3"],
                C12=plan["b2"]["C1"], C22=plan["b2"]["C2"], C32=plan["b2"]["C3"],
                PC=plan["PC"])
    key = (NB, tuple(meta["C11"]), tuple(meta["C21"]), tuple(meta["C31"]),
           tuple(meta["C12"]), tuple(meta["C22"]), tuple(meta["C32"]),
           tuple(meta["PC"]))
    if key not in _CACHE:
        _CACHE[key] = _build(meta)
    nc = _CACHE[key]

    gorder = plan["gorder"]
    NGB = len(plan["PC"])
    invc_full = 1.0 / np.maximum(plan["sizes"], 1.0)

    def padrow(col):
        p = np.zeros((1, 128), np.float32)
        p[0, col] = -1e9
        return p.astype(ml_dtypes.bfloat16)

    in_maps = []
    for r in range(R):
        m = {}
        for br, bp in ((1, plan["b1"]), (2, plan["b2"])):
            x = np.asarray(inputs[f"x{br}"], np.float32)
            ids = bp["node_at"][r]
            xT = np.zeros((7, NPAD), np.float32)
            valid = ids >= 0
            xT[:, valid] = x[ids[valid]].T
            m[f"x{br}T"] = xT
            m[f"ia{br}"] = _wrap16(bp["ia"][r])
            m[f"im{br}"] = bp["im"][r]
            m[f"ip{br}"] = _wrap16(plan[f"ip{br}"][r])
            xn = np.asarray(inputs[f"x_norm2_{br}"], np.float32)
            m[f"xn{br}T"] = np.ascontiguousarray(xn[gorder[r]].T)
        ic = np.zeros((P, NGB), np.float32)
        for gb in range(NGB):
            ic[:, gb] = invc_full[gorder[r, gb * P:(gb + 1) * P]]
        m["invc"] = ic
        for l in (1, 2, 3):
            m[f"Wa{l}"] = wf[l - 1]["Waug"]
            m[f"Qi{l}"] = wf[l - 1]["Qinv"]
            m[f"bc{l}"] = wf[l - 1]["bcol"]
        m["linW"] = np.asarray(inputs["linW"], np.float32)
        m["linb"] = np.tile(np.asarray(inputs["linb"], np.float32)[None, :], (P, 1))
        m["padrow1"] = padrow(127)
        m["padrow2"] = padrow(127)
        m["padrow3"] = padrow(63)
        in_maps.append(m)

    import os, time as _time
    trace = os.environ.get("GAT_TRACE") == "1"
    _t0 = _time.time()
    res = run_bass_kernel_spmd(nc, in_maps, core_ids=list(range(R)), trace=trace)
    global LAST_RES, LAST_RUN_S
    LAST_RES = res
    LAST_RUN_S = _time.time() - _t0

    o1 = np.zeros((N_GRAPHS, 64), np.float32)
    o2 = np.zeros((N_GRAPHS, 64), np.float32)
    for r in range(R):
        o1[gorder[r]] = res.results[r]["o1"]
        o2[gorder[r]] = res.results[r]["o2"]
    return o1, o2
